# revision 11
# baseline (speedup 1.0000x reference)
import sys
for p in ('/opt/trn_rl_repo', '/opt/pypackages'):
    if p not in sys.path:
        sys.path.insert(0, p)
import numpy as np
from concourse import bass, bacc, tile, mybir
from concourse import bass_utils

B, C, T, K = 4, 64, 4096, 4
NCORES = 8
OS = T // NCORES          # 512: per-core token slice (phase-1 o-slice == phase-2 t-slice)
BC = B * C                # 256
f32 = mybir.dt.float32
f16 = mybir.dt.float16
u32 = mybir.dt.uint32

_cache = {}


def _build_l1():
    """Phase 1, SPMD core j: from xT (hi/lo fp16) and W^T column slices compute
    qn_j [256,512] f32 (normalized q slice), k_j [256,512] f32, and the conv-folded
    value tables u_j [4b,4k,4m,128,64] f16 where u[b,k][m*128+p] = (conv_w_k @ v_b)^T row."""
    nc = bacc.Bacc("TRN2", target_bir_lowering=False, debug=False, num_devices=NCORES)
    XH = nc.dram_tensor("xh", [T, BC], f16, kind="ExternalInput").ap()
    XL = nc.dram_tensor("xl", [T, BC], f16, kind="ExternalInput").ap()
    WQH = nc.dram_tensor("wqh", [T, OS], f16, kind="ExternalInput").ap()
    WQL = nc.dram_tensor("wql", [T, OS], f16, kind="ExternalInput").ap()
    WKH = nc.dram_tensor("wkh", [T, OS], f16, kind="ExternalInput").ap()
    WKL = nc.dram_tensor("wkl", [T, OS], f16, kind="ExternalInput").ap()
    WV = nc.dram_tensor("wv", [T, OS], f16, kind="ExternalInput").ap()
    CW = nc.dram_tensor("cw", [2 * C, K * C], f16, kind="ExternalInput").ap()
    QN = nc.dram_tensor("qn_o", [BC, OS], f32, kind="ExternalOutput").ap()
    KO = nc.dram_tensor("k_o", [BC, OS], f32, kind="ExternalOutput").ap()
    UO = nc.dram_tensor("u_o", [B, K, 4, 128, C], f16, kind="ExternalOutput").ap()

    NKT = T // 128  # 32 contraction tiles

    with tile.TileContext(nc) as tc:
        with tc.tile_pool(name="xp", bufs=1) as xp, \
             tc.tile_pool(name="wp", bufs=6) as wp, \
             tc.tile_pool(name="sp", bufs=2) as sp, \
             tc.tile_pool(name="cp", bufs=1) as cp, \
             tc.tile_pool(name="pp", bufs=2, space="PSUM") as pp, \
             tc.tile_pool(name="pu", bufs=1, space="PSUM") as pu:
            xh = xp.tile([128, NKT, BC], f16, tag="xh")
            xl = xp.tile([128, NKT, BC], f16, tag="xl")
            for kt in range(NKT):
                nc.sync.dma_start(out=xh[:, kt, :],
                                  in_=XH[kt * 128:(kt + 1) * 128, :])
                nc.sync.dma_start(out=xl[:, kt, :],
                                  in_=XL[kt * 128:(kt + 1) * 128, :])
            cw = cp.tile([2 * C, K * C], f16, tag="cw")
            nc.sync.dma_start(out=cw[:, :], in_=CW[:, :])
            ones_r = cp.tile([128, 1], f32, tag="ones_r")   # reduce lhsT
            nc.vector.memset(ones_r[:, :], 1.0)
            ones_b = cp.tile([1, C], f32, tag="ones_b")   # broadcast lhsT
            nc.vector.memset(ones_b[:, :], 1.0)

            # ---- Q and K: split-3 fp16 matmuls; weights loaded once per kt ----
            qsb = {}   # mt -> f32 sbuf tile [128, OS] (raw q)
            ksb = {}
            for which, (WH, WL, store) in (("q", (WQH, WQL, qsb)),
                                           ("k", (WKH, WKL, ksb))):
                accs = []
                for mt in range(2):
                    acc_t = pp.tile([128, OS], f32, tag=f"acc{mt}")
                    accs.append(acc_t)
                for kt4 in range(NKT // 4):
                    wh = wp.tile([128, 4, OS], f16, tag=f"w{which}h")
                    wl = wp.tile([128, 4, OS], f16, tag=f"w{which}l")
                    for t in range(4):
                        r0 = (kt4 * 4 + t) * 128
                        nc.sync.dma_start(out=wh[:, t, :], in_=WH[r0:r0 + 128, :])
                        nc.sync.dma_start(out=wl[:, t, :], in_=WL[r0:r0 + 128, :])
                    for t in range(4):
                        kt = kt4 * 4 + t
                        first = (kt == 0)
                        last = (kt == NKT - 1)
                        for mt in range(2):
                            lh = xh[:, kt, mt * 128:(mt + 1) * 128]
                            ll = xl[:, kt, mt * 128:(mt + 1) * 128]
                            nc.tensor.matmul(out=accs[mt][:, :], lhsT=lh, rhs=wh[:, t, :],
                                             start=first, stop=False)
                            nc.tensor.matmul(out=accs[mt][:, :], lhsT=lh, rhs=wl[:, t, :],
                                             start=False, stop=False)
                            nc.tensor.matmul(out=accs[mt][:, :], lhsT=ll, rhs=wh[:, t, :],
                                             start=False, stop=last)
                for mt in range(2):
                    res = sp.tile([128, OS], f32, tag=f"{which}sb{mt}")
                    nc.scalar.copy(out=res[:, :], in_=accs[mt][:, :])
                    store[mt] = res

            # ---- V (single fp16 matmul) ----
            vsb = {}
            vaccs = []
            for mt in range(2):
                vacc_t = pp.tile([128, OS], f32, tag=f"acc{mt}")
                vaccs.append(vacc_t)
            for kt4 in range(NKT // 4):
                wv = wp.tile([128, 4, OS], f16, tag="wv")
                for t in range(4):
                    r0 = (kt4 * 4 + t) * 128
                    nc.sync.dma_start(out=wv[:, t, :], in_=WV[r0:r0 + 128, :])
                for t in range(4):
                    kt = kt4 * 4 + t
                    for mt in range(2):
                        lh = xh[:, kt, mt * 128:(mt + 1) * 128]
                        ll = xl[:, kt, mt * 128:(mt + 1) * 128]
                        nc.tensor.matmul(out=vaccs[mt][:, :], lhsT=lh, rhs=wv[:, t, :],
                                         start=(kt == 0), stop=False)
                        nc.tensor.matmul(out=vaccs[mt][:, :], lhsT=ll, rhs=wv[:, t, :],
                                         start=False, stop=(kt == NKT - 1))
            for mt in range(2):
                v16 = sp.tile([128, OS], f16, tag=f"vsb{mt}")
                nc.scalar.copy(out=v16[:, :], in_=vaccs[mt][:, :])
                vsb[mt] = v16

            # ---- U tables: u[b,k] = (v_b^T @ cw_k) as 4 M-tiles of [128, 64] ----
            for b in range(B):
                off = (b % 2) * C
                vt = vsb[b // 2][off:off + C, :]  # [64, 512] f16
                for k in range(K):
                    u16 = sp.tile([128, 4, C], f16, tag="u16")
                    for m in range(4):
                        pt = pu.tile([128, C], f32, tag="pu")
                        nc.tensor.matmul(out=pt[:, :],
                                         lhsT=vt[:, m * 128:(m + 1) * 128],
                                         rhs=cw[off:off + C, k * C:(k + 1) * C],
                                         start=True, stop=True)
                        nc.scalar.copy(out=u16[:, m, :], in_=pt[:, :])
                    for m in range(4):
                        nc.sync.dma_start(out=UO[b, k, m], in_=u16[:, m, :])

            # ---- qn = q / ||q||_col  (norm over the 64 channels of each batch) ----
            for b in range(B):
                off = (b % 2) * C
                q_b = qsb[b // 2][off:off + C, :]  # [64, 512] f32
                sq = sp.tile([128, OS], f32, tag="sq")
                nc.scalar.square(out=sq[off:off + C, :], in_=q_b)
                pn = pu.tile([1, OS], f32, tag="pn")
                nc.tensor.matmul(out=pn[:, :], lhsT=ones_r[off:off + C, :],
                                 rhs=sq[off:off + C, :], start=True, stop=True)
                nrm = sp.tile([1, OS], f32, tag="nrm")
                nc.scalar.sqrt(out=nrm[:, :], in_=pn[:, :])
                rcp = sp.tile([1, OS], f32, tag="rcp")
                nc.vector.reciprocal(out=rcp[:, :], in_=nrm[:, :])
                pb = pu.tile([128, OS], f32, tag="pb")
                nc.tensor.matmul(out=pb[off:off + C, :], lhsT=ones_b[:, :],
                                 rhs=rcp[:, :], start=True, stop=True)
                bc = sp.tile([128, OS], f32, tag="bc")
                nc.scalar.copy(out=bc[off:off + C, :], in_=pb[off:off + C, :])
                qn = sp.tile([128, OS], f32, tag="qn")
                nc.vector.tensor_mul(out=qn[off:off + C, :], in0=q_b,
                                     in1=bc[off:off + C, :])
                nc.sync.dma_start(out=QN[b * C:(b + 1) * C, :], in_=qn[off:off + C, :])

            for mt in range(2):
                nc.sync.dma_start(out=KO[mt * 128:(mt + 1) * 128, :],
                                  in_=ksb[mt][:, :])
    nc.compile()
    return nc


def _build_l2():
    """Phase 2, SPMD core j: rows t in [j*512,(j+1)*512) for all batches.
    sim = k_t . qn_s via fp16-split matmuls, exact top-4 via max/max_index,
    gather-sum of u tables -> yT, partial out = yT^T @ WoT slice."""
    nc = bacc.Bacc("TRN2", target_bir_lowering=False, debug=False, num_devices=NCORES)
    QN = nc.dram_tensor("qn", [BC, T], f32, kind="ExternalInput").ap()
    KJ = nc.dram_tensor("kj", [BC, OS], f32, kind="ExternalInput").ap()
    WOT = nc.dram_tensor("wot", [OS, T], f16, kind="ExternalInput").ap()
    UT = [[nc.dram_tensor(f"ut{b}_{k}", [T, C], f16, kind="ExternalInput").ap()
           for k in range(K)] for b in range(B)]
    OUT = nc.dram_tensor("out_o", [2, 128, T], f32, kind="ExternalOutput").ap()

    with tile.TileContext(nc) as tc:
        with tc.tile_pool(name="qp", bufs=1) as qp, \
             tc.tile_pool(name="wp", bufs=1) as wp, \
             tc.tile_pool(name="sp", bufs=2) as sp, \
             tc.tile_pool(name="simp", bufs=2) as simp, \
             tc.tile_pool(name="yp", bufs=1) as yp, \
             tc.tile_pool(name="pp", bufs=2, space="PSUM") as pp, \
             tc.tile_pool(name="po", bufs=2, space="PSUM") as po:
            # load + split qn into fp16 hi/lo
            qh, ql = {}, {}
            for mt in range(2):
                qf = qp.tile([128, T], f32, tag="qf")
                nc.sync.dma_start(out=qf[:, :], in_=QN[mt * 128:(mt + 1) * 128, :])
                h16 = qp.tile([128, T], f16, tag=f"qh{mt}")
                nc.scalar.copy(out=h16[:, :], in_=qf[:, :])
                h32 = qp.tile([128, T], f32, tag="h32")
                nc.scalar.copy(out=h32[:, :], in_=h16[:, :])
                l16 = qp.tile([128, T], f16, tag=f"ql{mt}")
                nc.vector.tensor_sub(out=l16[:, :], in0=qf[:, :], in1=h32[:, :])
                qh[mt], ql[mt] = h16, l16
            # k slice hi/lo
            kh, kl = {}, {}
            for mt in range(2):
                kf = sp.tile([128, OS], f32, tag="kf")
                nc.sync.dma_start(out=kf[:, :], in_=KJ[mt * 128:(mt + 1) * 128, :])
                h16 = qp.tile([128, OS], f16, tag=f"kh{mt}")
                nc.scalar.copy(out=h16[:, :], in_=kf[:, :])
                h32 = sp.tile([128, OS], f32, tag="kh32")
                nc.scalar.copy(out=h32[:, :], in_=h16[:, :])
                l16 = qp.tile([128, OS], f16, tag=f"kl{mt}")
                nc.vector.tensor_sub(out=l16[:, :], in0=kf[:, :], in1=h32[:, :])
                kh[mt], kl[mt] = h16, l16
            wot = {}
            for kt in range(4):
                w = wp.tile([128, T], f16, tag=f"wot{kt}")
                nc.sync.dma_start(out=w[:, :], in_=WOT[kt * 128:(kt + 1) * 128, :])
                wot[kt] = w

            ytp = {}  # (pair, kt) -> [128, 128] f16 lhsT tiles for the out matmul
            for pair in range(2):
                for kt in range(4):
                    yt_tile = yp.tile([128, 128], f16, tag=f"yt{pair}{kt}")
                    ytp[(pair, kt)] = yt_tile

            NCH = T // 512  # 8 s-chunks
            for b in range(B):
                khb = kh[b // 2][(b % 2) * C:(b % 2) * C + C, :]   # [64, 512] f16
                klb = kl[b // 2][(b % 2) * C:(b % 2) * C + C, :]
                qhb = qh[b // 2][(b % 2) * C:(b % 2) * C + C, :]   # [64, 4096] f16
                qlb = ql[b // 2][(b % 2) * C:(b % 2) * C + C, :]
                for i in range(4):  # 128-token blocks of this core's slice
                    lh = khb[:, i * 128:(i + 1) * 128]
                    ll = klb[:, i * 128:(i + 1) * 128]
                    sim = simp.tile([128, T], f32, tag="sim")
                    for ch in range(NCH):
                        ps = pp.tile([128, 512], f32, tag="ps")
                        rh = qhb[:, ch * 512:(ch + 1) * 512]
                        rl = qlb[:, ch * 512:(ch + 1) * 512]
                        nc.tensor.matmul(out=ps[:, :], lhsT=lh, rhs=rh,
                                         start=True, stop=False)
                        nc.tensor.matmul(out=ps[:, :], lhsT=ll, rhs=rh,
                                         start=False, stop=False)
                        nc.tensor.matmul(out=ps[:, :], lhsT=lh, rhs=rl,
                                         start=False, stop=True)
                        nc.scalar.copy(out=sim[:, ch * 512:(ch + 1) * 512],
                                       in_=ps[:, :])
                    m8 = sp.tile([128, 8], f32, tag="m8")
                    i8 = sp.tile([128, 8], u32, tag="i8")
                    nc.vector.max(out=m8[:, :], in_=sim[:, :])
                    nc.vector.max_index(out=i8[:, :], in_max=m8[:, :], in_values=sim[:, :])
                    gth = sp.tile([128, K, C], f16, tag="gth")
                    for k in range(K):
                        nc.gpsimd.indirect_dma_start(
                            out=gth[:, k, :], out_offset=None,
                            in_=UT[b][k][:, :],
                            in_offset=bass.IndirectOffsetOnAxis(ap=i8[:, k:k + 1], axis=0))
                    t0 = sp.tile([128, C], f16, tag="t0")
                    t1 = sp.tile([128, C], f16, tag="t1")
                    nc.gpsimd.tensor_add(out=t0[:, :], in0=gth[:, 0, :], in1=gth[:, 1, :])
                    nc.gpsimd.tensor_add(out=t1[:, :], in0=gth[:, 2, :], in1=gth[:, 3, :])
                    dst = ytp[(b // 2, i)][:, (b % 2) * C:(b % 2) * C + C]
                    nc.gpsimd.tensor_add(out=dst, in0=t0[:, :], in1=t1[:, :])

            # partial out: for batch pair, out[128(2b,c), T] = sum_kt ytp^T @ wot
            for pair in range(2):
                ob = simp.tile([128, T], f32, tag="ob")
                for ch in range(NCH):
                    ps = po.tile([128, 512], f32, tag="po")
                    for kt in range(4):
                        nc.tensor.matmul(out=ps[:, :], lhsT=ytp[(pair, kt)][:, :],
                                         rhs=wot[kt][:, ch * 512:(ch + 1) * 512],
                                         start=(kt == 0), stop=(kt == 3))
                    nc.scalar.copy(out=ob[:, ch * 512:(ch + 1) * 512], in_=ps[:, :])
                nc.sync.dma_start(out=OUT[pair], in_=ob[:, :])
    nc.compile()
    return nc


def _split16(a):
    h = a.astype(np.float16)
    l = (a - h.astype(np.float32)).astype(np.float16)
    return h, l


def kernel(x, Wq, Wk, Wv, Wo, conv_w, conv_b):
    x = np.asarray(x, np.float32)
    Wq = np.asarray(Wq, np.float32); Wk = np.asarray(Wk, np.float32)
    Wv = np.asarray(Wv, np.float32); Wo = np.asarray(Wo, np.float32)
    conv_w = np.asarray(conv_w, np.float32); conv_b = np.asarray(conv_b, np.float32)

    if "l1" not in _cache:
        _cache["l1"] = _build_l1()
    if "l2" not in _cache:
        _cache["l2"] = _build_l2()

    xT = np.ascontiguousarray(x.transpose(2, 0, 1).reshape(T, BC))  # [t, b*64+c]
    xh, xl = _split16(xT)
    WqT, WkT = Wq.T, Wk.T
    WvT16 = np.ascontiguousarray(Wv.T).astype(np.float16)
    cw1 = np.ascontiguousarray(conv_w.transpose(1, 2, 0).reshape(C, K * C)).astype(np.float16)
    cw = np.concatenate([cw1, cw1], axis=0)
    # cw[ci, k*64+co] = conv_w[co, ci, k]

    in_maps = []
    for j in range(NCORES):
        sl = slice(j * OS, (j + 1) * OS)
        wqh, wql = _split16(np.ascontiguousarray(WqT[:, sl]))
        wkh, wkl = _split16(np.ascontiguousarray(WkT[:, sl]))
        in_maps.append({"xh": xh, "xl": xl, "wqh": wqh, "wql": wql,
                        "wkh": wkh, "wkl": wkl,
                        "wv": np.ascontiguousarray(WvT16[:, sl]), "cw": cw})
    r1 = bass_utils.run_bass_kernel_spmd(_cache["l1"], in_maps, core_ids=list(range(NCORES)))

    qn = np.concatenate([r1.results[j]["qn_o"] for j in range(NCORES)], axis=1)  # [256, T]
    kf = np.concatenate([r1.results[j]["k_o"] for j in range(NCORES)], axis=1)   # [256, T]
    ut = {}
    for b in range(B):
        for k in range(K):
            ut[(b, k)] = np.ascontiguousarray(np.concatenate(
                [r1.results[j]["u_o"][b, k].reshape(OS, C) for j in range(NCORES)], axis=0))

    in_maps2 = []
    for j in range(NCORES):
        sl = slice(j * OS, (j + 1) * OS)
        m = {"qn": qn, "kj": np.ascontiguousarray(kf[:, sl]),
             "wot": np.ascontiguousarray(Wo.T[sl, :]).astype(np.float16)}
        for b in range(B):
            for k in range(K):
                m[f"ut{b}_{k}"] = ut[(b, k)]
        in_maps2.append(m)
    r2 = bass_utils.run_bass_kernel_spmd(_cache["l2"], in_maps2, core_ids=list(range(NCORES)))

    out = np.zeros((B, C, T), np.float32)
    for j in range(NCORES):
        oo = r2.results[j]["out_o"]  # [2, 128, T]
        for b in range(B):
            out[b] += oo[b // 2, (b % 2) * C:(b % 2) * C + C, :]
    bias = conv_b[:, None] * Wo.sum(axis=1)[None, :]  # [C, T_out? ] -> [64, 4096]
    out += bias[None, :, :]
    return out
